# revision 41
# baseline (speedup 1.0000x reference)
"""ExpertsChooseMlp Trainium2 kernel — all-fp8 DoubleRow pipeline.

Full inputs in, full output out. Sharding: 8 cores = 4 batches x 2 expert-pairs.
Core m handles batch b=m//2 and experts {2g, 2g+1}, g=m%2. Each core computes
pout[T,O] = sum_{e in pair} combine[b,:,e,:] @ mlp_e(dispatch[b,:,e,:]^T @ x[b]);
the host sums the two partials per batch and adds b2 + rank-1 corrections.

All four matmul phases run as fp8-e4m3 DoubleRow (K=256/pass, ~1.75x bf16):
  xdT[D,C] = matmul(lhsT=x8[T,D],   rhs=u8[T,C],  DR)   (K=T)
  hT[HE,C] = matmul(lhsT=w18[D,HE], rhs=xdT[D,C], DR)   (K=D), GELU + beta
  y[C,O]   = matmul(lhsT=hT[HE,C],  rhs=w28[HE,O],DR)   (K=HE)
  pout[T,O]= matmul(lhsT=cmT[C,T],  rhs=y[C,O],   DR)   (K=C)

fp8 error control (fp32 reference absmax-rel err ~8.7e-3, budget 2e-2):
fp8 quantization errors of per-(t,c) data average out across the capacity
reduction, but errors in operands SHARED across C (x, w1, w2) are correlated
and do not. Three rank-1 corrections cancel those correlated components:
  1. dispatch mask is mean-shifted: u = dm - 0.5 quantized to fp8 (halves the
     mask quant error); the c-independent term v[d] = 0.5*sum_t x[t,d] is
     computed host-side from EXACT fp32 x, killing the rank-1 part of x's
     quant error.
  2. v is folded through fc1 with EXACT fp32 w1: beta = w1^T v + b1 becomes
     the per-partition activation bias, killing the rank-1 part of w1's
     quant error.
  3. w2's correlated error mean_c(h)^T (w2 - w28) is cancelled host-side
     with mu_h = E_c[h] evaluated analytically (pre-GELU activations are
     ~N(beta, s2) across c; E[gelu(N(beta,s2))] has a closed form), adding
     outer(rowsum_cm, mu_h @ ew2) to the output.

Schedule notes (trace-driven; measured 88.0-88.4us HW exec, vs 128.9us for
the bf16 baseline and a ~69us pure-matmul floor at the fp8 peak):
  - fp8 DR matmul streams N=512 in ~216ns (the 157 TF/s fp8 peak);
    LDWEIGHTS fully hides under the previous matmul's streaming.
  - All HBM operands are pre-arranged host-side into the exact SBUF
    DoubleRow plane layouts, so every DMA is a big contiguous burst
    (strided gather descriptors measured only ~25-35 GB/s/queue).
  - Only 2 HWDGE queues exist (sync + scalar engines ring their paced
    doorbells, which OCCUPIES the engine until its transfers finish);
    x/dm interleave across both in consumption order — one queue cannot
    sustain phase A's 148 GB/s arrival rate during the DMA ramp. cmt
    queues behind everything (needed only by phase D).
  - Stage order A0,A1,B0,C0,B1,C1,D: A0's PSUM copies hide under A1,
    activations get a full stage of slack. Phase A runs a single kp pass
    with all 8 PSUM banks (round-splitting doubles the dm arrival rate).
    PSUM->SBUF copies are issued in BANK order = consumer order.
  - pout is written bf16 (f32's 4MB cannot drain inside phase D on one
    ~125 GB/s write queue; splitting across queues contends DOWN), with
    drain copies alternating scalar/vector (vector is busy with C1's
    y-copies when the drain starts) and the last 4 chunks split across
    both queues to shorten the end-of-kernel chain.
  - Warmup matmuls (pstate ramp + DMA-wait cover) start off a vector-engine
    memset; the gpsimd memset path took ~4us to dispatch. gpsimd cannot
    read PSUM, and its SWDGE steals HBM bandwidth if used during the
    critical dm window.
"""
import sys

sys.path.insert(0, "/opt/trn_rl_repo")

import numpy as np
import ml_dtypes

import concourse.bacc as bacc
import concourse.mybir as mybir
import concourse.tile as tile
from concourse import bass_utils

B, T, D, E, C, HE, O = 4, 2048, 512, 4, 1024, 512, 512
P = 128
nKP = T // (2 * P)   # 8  T pair-chunks (DR: K=256 per matmul)
nMD = D // P         # 4  D-chunks
nMH = HE // P        # 4  HE-chunks
nKD2 = D // (2 * P)  # 2  D pair-chunks
nCC = C // P         # 8  C-chunks
nKH2 = HE // (2 * P) # 2  HE pair-chunks
nMT = T // P         # 16
NF = 512             # matmul free dim (one PSUM bank)
nCP = nCC // 2       # 4  C pair-chunks for combine

F32 = mybir.dt.float32
BF16 = mybir.dt.bfloat16
F8 = mybir.dt.float8e4
GELU = mybir.ActivationFunctionType.Gelu
DR = mybir.MatmulPerfMode.DoubleRow
DM_SHIFT = 0.5

_NC = None


def _build():
    nc = bacc.Bacc("TRN2", target_bir_lowering=False, debug=False,
                   enable_asserts=False, num_devices=1)
    # All inputs pre-arranged host-side into SBUF plane layouts (contiguous).
    xb = nc.dram_tensor("xb", [nKP, P, 2, D], F8, kind="ExternalInput").ap()
    dm = nc.dram_tensor("dm", [2, nKP, P, 2, C], F8, kind="ExternalInput").ap()
    cmt = nc.dram_tensor("cmt", [2, nCP, P, 2, T], F8, kind="ExternalInput").ap()
    w1 = nc.dram_tensor("w1", [P, 2, nKD2, 2, HE], F8, kind="ExternalInput").ap()
    w2 = nc.dram_tensor("w2", [P, 2, nKH2, 2, O], F8, kind="ExternalInput").ap()
    beta = nc.dram_tensor("beta", [P, 2 * nMH], F32, kind="ExternalInput").ap()
    # pout in bf16: one HWDGE queue writes ~125 GB/s (splitting across queues
    # contends DOWN to ~105), so f32's 4MB can't drain inside phase D's 27us
    # — bf16's 2MB can. Costs ~+3e-3 absmax-rel worst case.
    pout = nc.dram_tensor("pout", [T, O], BF16, kind="ExternalOutput").ap()

    with tile.TileContext(nc) as tc:
        with (
            tc.tile_pool(name="const", bufs=1) as const,
            tc.tile_pool(name="dmp", bufs=16) as dmp,
            tc.tile_pool(name="cmp", bufs=8) as cmp_,
            tc.tile_pool(name="inter", bufs=2) as inter,
            tc.tile_pool(name="yp", bufs=2) as yp,
            tc.tile_pool(name="outp", bufs=2) as outp,
            tc.tile_pool(name="psum", bufs=8, space="PSUM") as psp,
        ):
            # ---- DMA plan (3 queues: scalar + sync HWDGE, gpsimd SWDGE) ----
            # scalar: x (needed t=0..14us), w1+beta (t~14), w2 (t~31).
            # sync:   dm e0 (t=0..14), dm e1 (t~18..31).
            # gpsimd: cmt e0+e1 (needed t~59+).
            # pout chunks are split into partition halves rotated over all 3.
            # The consumption-order-critical loads (x chunks + dm tiles)
            # alternate across BOTH HWDGE queues in consumption order: one
            # queue alone cannot sustain phase A's arrival rate during the
            # early DMA ramp.
            # The consumption-order-critical loads (x chunks + dm tiles)
            # alternate across BOTH HWDGE queues in consumption order: one
            # queue alone cannot sustain phase A's arrival rate during the
            # early DMA ramp. cmt (needed only by phase D) queues behind.
            x_sb = const.tile([P, nKP, 2, D], F8)
            dm_t = {}
            for ei in range(2):
                for kp in range(nKP):
                    t_ = dmp.tile([P, 2, C], F8, tag="dm")
                    dm_t[(ei, kp)] = t_
            for kp in range(nKP):
                qa = nc.sync if kp % 2 == 0 else nc.scalar
                qb = nc.scalar if kp % 2 == 0 else nc.sync
                qa.dma_start(dm_t[(0, kp)][:], dm[0, kp])
                qb.dma_start(x_sb[:, kp, :, :], xb[kp])
            w1_sb = const.tile([P, 2, nKD2, 2, HE], F8)
            nc.scalar.dma_start(w1_sb[:], w1[:])
            beta_sb = const.tile([P, 2 * nMH], F32)
            nc.scalar.dma_start(beta_sb[:], beta[:])
            for kp in range(nKP):
                eng = nc.sync if kp % 2 == 0 else nc.scalar
                eng.dma_start(dm_t[(1, kp)][:], dm[1, kp])
            w2_sb = const.tile([P, 2, nKH2, 2, O], F8)
            nc.scalar.dma_start(w2_sb[:], w2[:])
            cmt_t = {}
            for ei in range(2):
                for kp in range(nCP):
                    t_ = cmp_.tile([P, 2, T], F8, tag="cmt")
                    cmt_t[(ei, kp)] = t_
            for kp in range(nCP):
                nc.scalar.dma_start(cmt_t[(0, kp)][:], cmt[0, kp])
            for kp in range(nCP):
                nc.sync.dma_start(cmt_t[(1, kp)][:], cmt[1, kp])

            # ---- HAM warmup: dummy matmuls on a vector-memset tile during
            # the initial DMA wait, so real matmuls start at 2.4GHz. ----
            warm = const.tile([P, NF], BF16)
            nc.vector.memset(warm[:], 0.0)
            ps_w = psp.tile([P, NF], F32, tag="ps", name="ps_warm")
            for i in range(7):
                nc.tensor.matmul(ps_w[:], warm[:, 0:P], warm[:],
                                 start=(i == 0), stop=(i == 6))

            xdt = {}

            def stage_A(ei):
                # xdT[D, C] = x8^T u8, fp8 DR. Single pass over kp with all
                # 8 PSUM banks: each dm tile is consumed once at 148 GB/s
                # (any round-split doubles the required DMA arrival rate —
                # dm tiles stream in just-in-time for BOTH experts).
                # Copies in BANK order = the next stage's consumption order.
                xdt[ei] = inter.tile([P, nKD2, 2, C], F8, tag="xdt",
                                     name=f"xdt{ei}")
                pss = [psp.tile([P, NF], F32, tag="ps", name=f"psa{i}")
                       for i in range(2 * nMD)]
                for kp in range(nKP):
                    for mc in range(nMD):
                        lhsT = x_sb[:, kp, :, mc * P:(mc + 1) * P]
                        for ncc in range(2):
                            nc.tensor.matmul(
                                pss[2 * mc + ncc][:], lhsT,
                                dm_t[(ei, kp)][:, :, ncc * NF:(ncc + 1) * NF],
                                start=(kp == 0), stop=(kp == nKP - 1),
                                perf_mode=DR)
                for j in range(2 * nMD):
                    mc, ncc = j // 2, j % 2
                    nc.vector.tensor_copy(
                        xdt[ei][:, mc // 2, mc % 2,
                                ncc * NF:(ncc + 1) * NF],
                        pss[j][:])

            ht = {}

            def stage_B(ei):
                # hT[HE, C] = gelu(w18^T xdT + beta), fp8 DR.
                ht[ei] = inter.tile([P, nKH2, 2, C], F8, tag="ht",
                                    name=f"ht{ei}")
                for ncc in range(2):
                    sl = slice(ncc * NF, (ncc + 1) * NF)
                    for mh in range(nMH):
                        ps0 = psp.tile([P, NF], F32, tag="ps")
                        for kd2 in range(nKD2):
                            nc.tensor.matmul(
                                ps0[:],
                                w1_sb[:, ei, kd2, :, mh * P:(mh + 1) * P],
                                xdt[ei][:, kd2, :, sl],
                                start=(kd2 == 0), stop=(kd2 == nKD2 - 1),
                                perf_mode=DR)
                        bia = beta_sb[:, ei * nMH + mh:ei * nMH + mh + 1]
                        nc.scalar.activation(ht[ei][:, mh // 2, mh % 2, sl],
                                             ps0[:], GELU, bias=bia)

            y_tiles = {}

            def stage_C(ei):
                # y[C, O] = hT^T w28, fp8 DR (DoubleRow plane layout for
                # phase D: row c = cp*256 + i*128 + p).
                y_sb = yp.tile([P, nCP, 2, O], F8, tag="y")
                for cc in range(nCC):
                    ps = psp.tile([P, NF], F32, tag="ps")
                    for kh2 in range(nKH2):
                        nc.tensor.matmul(
                            ps[:],
                            ht[ei][:, kh2, :, cc * P:(cc + 1) * P],
                            w2_sb[:, ei, kh2, :, :],
                            start=(kh2 == 0), stop=(kh2 == nKH2 - 1),
                            perf_mode=DR)
                    nc.vector.tensor_copy(y_sb[:, cc // 2, cc % 2, :], ps[:])
                y_tiles[ei] = y_sb

            # Stage order: every PSUM-copy / activation dependency gets a
            # full matmul stage of slack before its consumer (A0's copies
            # hide under A1, B0's activations under C0, etc).
            stage_A(0)
            stage_A(1)
            stage_B(0)
            stage_C(0)
            stage_B(1)
            stage_C(1)

            # ---- phase D: pout[T, O] = sum_e cmT_e^T y_e (fp8 DR) ----
            for mt in range(nMT):
                ps = psp.tile([P, NF], F32, tag="ps")
                idx = 0
                for ei in range(2):
                    for kp in range(nCP):
                        nc.tensor.matmul(ps[:],
                                         cmt_t[(ei, kp)][:, :, mt * P:(mt + 1) * P],
                                         y_tiles[ei][:, kp, :, :],
                                         start=(idx == 0), stop=(idx == 7),
                                         perf_mode=DR)
                        idx += 1
                ot = outp.tile([P, O], BF16, tag="out")
                # alternate copy engines: vector is busy with C1's y-copies
                # when the drain starts, which otherwise delays it ~5us
                if mt % 2 == 0:
                    nc.scalar.activation(ot[:], ps[:],
                                         mybir.ActivationFunctionType.Copy)
                else:
                    nc.vector.tensor_copy(ot[:], ps[:])
                if mt < nMT - 4:
                    nc.sync.dma_start(pout[mt * P:(mt + 1) * P, :], ot[:])
                else:
                    # last chunks: split across both queues to shorten the
                    # end-of-kernel drain chain
                    h = P // 2
                    nc.sync.dma_start(pout[mt * P:mt * P + h, :], ot[0:h, :])
                    nc.scalar.dma_start(pout[mt * P + h:(mt + 1) * P, :],
                                        ot[h:P, :])

    nc.compile()
    return nc


def get_nc():
    global _NC
    if _NC is None:
        _NC = _build()
    return _NC


def make_in_maps(x, dispatch_mask, combine_array, w1, b1, w2):
    f8 = ml_dtypes.float8_e4m3
    in_maps = []
    # x in DR plane layout [nKP, P, 2, D], t = kp*256 + i*128 + p
    x8_by_b = [
        np.ascontiguousarray(
            x[b].reshape(nKP, 2, P, D).transpose(0, 2, 1, 3)).astype(f8)
        for b in range(B)]
    w18 = w1.astype(f8)
    w28 = w2.astype(f8)
    for m in range(8):
        b, g = m // 2, m % 2
        es = slice(2 * g, 2 * g + 2)
        # dm -> [2, nKP, P, 2, C] (shifted), t = kp*256 + i*128 + p
        dm_s = np.ascontiguousarray(
            (np.transpose(dispatch_mask[b, :, es, :], (1, 0, 2)) - DM_SHIFT)
            .reshape(2, nKP, 2, P, C).transpose(0, 1, 3, 2, 4)).astype(f8)
        # cmT -> [2, nCP, P, 2, T], c = cp*256 + i*128 + p
        cmt_s = np.ascontiguousarray(
            np.transpose(combine_array[b, :, es, :], (1, 2, 0))
            .reshape(2, nCP, 2, P, T).transpose(0, 1, 3, 2, 4)).astype(f8)
        # w1 -> [P, 2, nKD2, 2, HE], d = kd2*256 + i*128 + p
        w1_s = np.ascontiguousarray(
            w18[es].reshape(2, nKD2, 2, P, HE).transpose(3, 0, 1, 2, 4))
        # w2 -> [P, 2, nKH2, 2, O], h' = kh2*256 + i*128 + p
        w2_s = np.ascontiguousarray(
            w28[es].reshape(2, nKH2, 2, P, O).transpose(3, 0, 1, 2, 4))
        # beta = w1^T v + b1 in fp32 with EXACT x and w1 (kills the rank-1
        # component of the x / w1 fp8 quantization errors)
        v = DM_SHIFT * x[b].sum(axis=0)                      # [D]
        beta = np.einsum("edh,d->eh", w1[es], v) + b1[es]    # [2, HE]
        beta_s = np.ascontiguousarray(
            beta.reshape(2, nMH, P).transpose(2, 0, 1).reshape(P, 2 * nMH))
        in_maps.append({
            "xb": x8_by_b[b],
            "dm": dm_s,
            "cmt": cmt_s,
            "w1": w1_s,
            "w2": w2_s,
            "beta": beta_s.astype(np.float32),
        })
    return in_maps


def _norm_cdf(z):
    from math import erf
    return 0.5 * (1.0 + np.array([erf(v / np.sqrt(2.0)) for v in z],
                                 dtype=np.float64))


def kernel(x, dispatch_mask, combine_array, w1, b1, w2, b2):
    nc = get_nc()
    x, dispatch_mask, combine_array, w1, b1, w2 = (
        np.asarray(a, dtype=np.float32)
        for a in (x, dispatch_mask, combine_array, w1, b1, w2))
    in_maps = make_in_maps(x, dispatch_mask, combine_array, w1, b1, w2)
    res = bass_utils.run_bass_kernel_spmd(nc, in_maps, core_ids=list(range(8)))
    b2f = np.asarray(b2, dtype=np.float32)
    f8 = ml_dtypes.float8_e4m3
    w1q = w1.astype(f8).astype(np.float32)
    w2q = w2.astype(f8).astype(np.float32)
    ew2 = w2 - w2q                                           # [E, HE, O]
    xq = x.astype(f8).astype(np.float32)
    uq = (dispatch_mask - DM_SHIFT).astype(f8).astype(np.float32)
    out = np.empty((B, T, O), dtype=np.float32)
    for b in range(B):
        p0 = np.asarray(res.results[2 * b]["pout"], dtype=np.float32)
        p1 = np.asarray(res.results[2 * b + 1]["pout"], dtype=np.float32)
        acc = p0 + p1 + b2f
        # w2-quantization rank-1 correction per expert:
        #   out += outer(rowsum_cm, mu_h @ ew2)  with mu_h = E_c[h] estimated
        # analytically: pre-GELU activations are ~N(beta, s2) across c, so
        # mu_h = E[gelu(N(beta, s2))] in closed form (Gaussian integral).
        rs_cm = combine_array[b].sum(axis=2)                 # [T, E]
        v = DM_SHIFT * x[b].sum(axis=0)
        vu = uq[b].var(axis=2)                               # [T, E]
        for e in range(E):
            beta = (w1[e].T @ v + b1[e]).astype(np.float64)  # [HE]
            s2 = (w1q[e] ** 2).T @ ((xq[b] ** 2).T @ vu[:, e])
            s2 = s2.astype(np.float64)
            zr = beta / np.sqrt(1.0 + s2)
            phi = np.exp(-0.5 * zr * zr) / np.sqrt(2.0 * np.pi)
            mu = beta * _norm_cdf(zr) + s2 / np.sqrt(1.0 + s2) * phi
            acc += np.outer(rs_cm[:, e],
                            mu.astype(np.float32) @ ew2[e])
        out[b] = acc
    return out


# revision 42
# speedup vs baseline: 1.0033x; 1.0033x over previous
"""ExpertsChooseMlp Trainium2 kernel — all-fp8 DoubleRow pipeline.

Full inputs in, full output out. Sharding: 8 cores = 4 batches x 2 expert-pairs.
Core m handles batch b=m//2 and experts {2g, 2g+1}, g=m%2. Each core computes
pout[T,O] = sum_{e in pair} combine[b,:,e,:] @ mlp_e(dispatch[b,:,e,:]^T @ x[b]);
the host sums the two partials per batch and adds b2 + rank-1 corrections.

All four matmul phases run as fp8-e4m3 DoubleRow (K=256/pass, ~1.75x bf16):
  xdT[D,C] = matmul(lhsT=x8[T,D],   rhs=u8[T,C],  DR)   (K=T)
  hT[HE,C] = matmul(lhsT=w18[D,HE], rhs=xdT[D,C], DR)   (K=D), GELU + beta
  y[C,O]   = matmul(lhsT=hT[HE,C],  rhs=w28[HE,O],DR)   (K=HE)
  pout[T,O]= matmul(lhsT=cmT[C,T],  rhs=y[C,O],   DR)   (K=C)

fp8 error control (fp32 reference absmax-rel err ~8.7e-3, budget 2e-2):
fp8 quantization errors of per-(t,c) data average out across the capacity
reduction, but errors in operands SHARED across C (x, w1, w2) are correlated
and do not. Three rank-1 corrections cancel those correlated components:
  1. dispatch mask is mean-shifted: u = dm - 0.5 quantized to fp8 (halves the
     mask quant error); the c-independent term v[d] = 0.5*sum_t x[t,d] is
     computed host-side from EXACT fp32 x, killing the rank-1 part of x's
     quant error.
  2. v is folded through fc1 with EXACT fp32 w1: beta = w1^T v + b1 becomes
     the per-partition activation bias, killing the rank-1 part of w1's
     quant error.
  3. w2's correlated error mean_c(h)^T (w2 - w28) is cancelled host-side
     with mu_h = E_c[h] evaluated analytically (pre-GELU activations are
     ~N(beta, s2) across c; E[gelu(N(beta,s2))] has a closed form), adding
     outer(rowsum_cm, mu_h @ ew2) to the output.

Schedule notes (trace-driven; measured 88.0-88.4us HW exec, vs 128.9us for
the bf16 baseline and a ~69us pure-matmul floor at the fp8 peak):
  - fp8 DR matmul streams N=512 in ~216ns (the 157 TF/s fp8 peak);
    LDWEIGHTS fully hides under the previous matmul's streaming.
  - All HBM operands are pre-arranged host-side into the exact SBUF
    DoubleRow plane layouts, so every DMA is a big contiguous burst
    (strided gather descriptors measured only ~25-35 GB/s/queue).
  - Only 2 HWDGE queues exist (sync + scalar engines ring their paced
    doorbells, which OCCUPIES the engine until its transfers finish);
    x/dm interleave across both in consumption order — one queue cannot
    sustain phase A's 148 GB/s arrival rate during the DMA ramp. cmt
    queues behind everything (needed only by phase D).
  - Stage order A0,A1,B0,C0,B1,C1,D: A0's PSUM copies hide under A1,
    activations get a full stage of slack. Phase A runs a single kp pass
    with all 8 PSUM banks (round-splitting doubles the dm arrival rate).
    PSUM->SBUF copies are issued in BANK order = consumer order.
  - pout is written bf16 (f32's 4MB cannot drain inside phase D on one
    ~125 GB/s write queue; splitting across queues contends DOWN), with
    drain copies alternating scalar/vector (vector is busy with C1's
    y-copies when the drain starts) and the last 4 chunks split across
    both queues to shorten the end-of-kernel chain.
  - Warmup matmuls (pstate ramp + DMA-wait cover) start off a vector-engine
    memset; the gpsimd memset path took ~4us to dispatch. gpsimd cannot
    read PSUM, and its SWDGE steals HBM bandwidth if used during the
    critical dm window.
"""
import sys

sys.path.insert(0, "/opt/trn_rl_repo")

import numpy as np
import ml_dtypes

import concourse.bacc as bacc
import concourse.mybir as mybir
import concourse.tile as tile
from concourse import bass_utils

B, T, D, E, C, HE, O = 4, 2048, 512, 4, 1024, 512, 512
P = 128
nKP = T // (2 * P)   # 8  T pair-chunks (DR: K=256 per matmul)
nMD = D // P         # 4  D-chunks
nMH = HE // P        # 4  HE-chunks
nKD2 = D // (2 * P)  # 2  D pair-chunks
nCC = C // P         # 8  C-chunks
nKH2 = HE // (2 * P) # 2  HE pair-chunks
nMT = T // P         # 16
NF = 512             # matmul free dim (one PSUM bank)
nCP = nCC // 2       # 4  C pair-chunks for combine

F32 = mybir.dt.float32
BF16 = mybir.dt.bfloat16
F8 = mybir.dt.float8e4
GELU = mybir.ActivationFunctionType.Gelu
DR = mybir.MatmulPerfMode.DoubleRow
DM_SHIFT = 0.5

_NC = None


def _build():
    nc = bacc.Bacc("TRN2", target_bir_lowering=False, debug=False,
                   enable_asserts=False, num_devices=1)
    # All inputs pre-arranged host-side into SBUF plane layouts (contiguous).
    xb = nc.dram_tensor("xb", [nKP, P, 2, D], F8, kind="ExternalInput").ap()
    dm = nc.dram_tensor("dm", [2, nKP, P, 2, C], F8, kind="ExternalInput").ap()
    cmt = nc.dram_tensor("cmt", [2, nCP, P, 2, T], F8, kind="ExternalInput").ap()
    w1 = nc.dram_tensor("w1", [P, 2, nKD2, 2, HE], F8, kind="ExternalInput").ap()
    w2 = nc.dram_tensor("w2", [P, 2, nKH2, 2, O], F8, kind="ExternalInput").ap()
    beta = nc.dram_tensor("beta", [P, 2 * nMH], F32, kind="ExternalInput").ap()
    # pout in bf16: one HWDGE queue writes ~125 GB/s (splitting across queues
    # contends DOWN to ~105), so f32's 4MB can't drain inside phase D's 27us
    # — bf16's 2MB can. Costs ~+3e-3 absmax-rel worst case.
    pout = nc.dram_tensor("pout", [T, O], BF16, kind="ExternalOutput").ap()

    with tile.TileContext(nc) as tc:
        with (
            tc.tile_pool(name="const", bufs=1) as const,
            tc.tile_pool(name="dmp", bufs=16) as dmp,
            tc.tile_pool(name="cmp", bufs=8) as cmp_,
            tc.tile_pool(name="inter", bufs=2) as inter,
            tc.tile_pool(name="yp", bufs=2) as yp,
            tc.tile_pool(name="outp", bufs=2) as outp,
            tc.tile_pool(name="psum", bufs=8, space="PSUM") as psp,
        ):
            # ---- DMA plan (3 queues: scalar + sync HWDGE, gpsimd SWDGE) ----
            # scalar: x (needed t=0..14us), w1+beta (t~14), w2 (t~31).
            # sync:   dm e0 (t=0..14), dm e1 (t~18..31).
            # gpsimd: cmt e0+e1 (needed t~59+).
            # pout chunks are split into partition halves rotated over all 3.
            # The consumption-order-critical loads (x chunks + dm tiles)
            # alternate across BOTH HWDGE queues in consumption order: one
            # queue alone cannot sustain phase A's arrival rate during the
            # early DMA ramp.
            # The consumption-order-critical loads (x chunks + dm tiles)
            # alternate across BOTH HWDGE queues in consumption order: one
            # queue alone cannot sustain phase A's arrival rate during the
            # early DMA ramp. cmt (needed only by phase D) queues behind.
            x_sb = const.tile([P, nKP, 2, D], F8)
            dm_t = {}
            for ei in range(2):
                for kp in range(nKP):
                    t_ = dmp.tile([P, 2, C], F8, tag="dm")
                    dm_t[(ei, kp)] = t_
            for kp in range(nKP):
                qa = nc.sync if kp % 2 == 0 else nc.scalar
                qb = nc.scalar if kp % 2 == 0 else nc.sync
                qa.dma_start(dm_t[(0, kp)][:], dm[0, kp])
                qb.dma_start(x_sb[:, kp, :, :], xb[kp])
            w1_sb = const.tile([P, 2, nKD2, 2, HE], F8)
            nc.scalar.dma_start(w1_sb[:], w1[:])
            beta_sb = const.tile([P, 2 * nMH], F32)
            nc.scalar.dma_start(beta_sb[:], beta[:])
            for kp in range(nKP):
                eng = nc.sync if kp % 2 == 0 else nc.scalar
                eng.dma_start(dm_t[(1, kp)][:], dm[1, kp])
            w2_sb = const.tile([P, 2, nKH2, 2, O], F8)
            nc.scalar.dma_start(w2_sb[:], w2[:])
            cmt_t = {}
            for ei in range(2):
                for kp in range(nCP):
                    t_ = cmp_.tile([P, 2, T], F8, tag="cmt")
                    cmt_t[(ei, kp)] = t_
            for kp in range(nCP):
                nc.scalar.dma_start(cmt_t[(0, kp)][:], cmt[0, kp])
            for kp in range(nCP):
                nc.sync.dma_start(cmt_t[(1, kp)][:], cmt[1, kp])

            # ---- HAM warmup: dummy matmuls on a vector-memset tile during
            # the initial DMA wait, so real matmuls start at 2.4GHz. ----
            warm = const.tile([P, NF], BF16)
            nc.vector.memset(warm[:], 0.0)
            ps_w = psp.tile([P, NF], F32, tag="ps", name="ps_warm")
            for i in range(8):
                nc.tensor.matmul(ps_w[:], warm[:, 0:P], warm[:],
                                 start=(i == 0), stop=(i == 7))

            xdt = {}

            def stage_A(ei):
                # xdT[D, C] = x8^T u8, fp8 DR. Single pass over kp with all
                # 8 PSUM banks: each dm tile is consumed once at 148 GB/s
                # (any round-split doubles the required DMA arrival rate —
                # dm tiles stream in just-in-time for BOTH experts).
                # Copies in BANK order = the next stage's consumption order.
                xdt[ei] = inter.tile([P, nKD2, 2, C], F8, tag="xdt",
                                     name=f"xdt{ei}")
                pss = [psp.tile([P, NF], F32, tag="ps", name=f"psa{i}")
                       for i in range(2 * nMD)]
                for kp in range(nKP):
                    for mc in range(nMD):
                        lhsT = x_sb[:, kp, :, mc * P:(mc + 1) * P]
                        for ncc in range(2):
                            nc.tensor.matmul(
                                pss[2 * mc + ncc][:], lhsT,
                                dm_t[(ei, kp)][:, :, ncc * NF:(ncc + 1) * NF],
                                start=(kp == 0), stop=(kp == nKP - 1),
                                perf_mode=DR)
                for j in range(2 * nMD):
                    mc, ncc = j // 2, j % 2
                    nc.vector.tensor_copy(
                        xdt[ei][:, mc // 2, mc % 2,
                                ncc * NF:(ncc + 1) * NF],
                        pss[j][:])

            ht = {}

            def stage_B(ei):
                # hT[HE, C] = gelu(w18^T xdT + beta), fp8 DR.
                ht[ei] = inter.tile([P, nKH2, 2, C], F8, tag="ht",
                                    name=f"ht{ei}")
                for ncc in range(2):
                    sl = slice(ncc * NF, (ncc + 1) * NF)
                    for mh in range(nMH):
                        ps0 = psp.tile([P, NF], F32, tag="ps")
                        for kd2 in range(nKD2):
                            nc.tensor.matmul(
                                ps0[:],
                                w1_sb[:, ei, kd2, :, mh * P:(mh + 1) * P],
                                xdt[ei][:, kd2, :, sl],
                                start=(kd2 == 0), stop=(kd2 == nKD2 - 1),
                                perf_mode=DR)
                        bia = beta_sb[:, ei * nMH + mh:ei * nMH + mh + 1]
                        nc.scalar.activation(ht[ei][:, mh // 2, mh % 2, sl],
                                             ps0[:], GELU, bias=bia)

            y_tiles = {}

            def stage_C(ei):
                # y[C, O] = hT^T w28, fp8 DR (DoubleRow plane layout for
                # phase D: row c = cp*256 + i*128 + p).
                y_sb = yp.tile([P, nCP, 2, O], F8, tag="y")
                for cc in range(nCC):
                    ps = psp.tile([P, NF], F32, tag="ps")
                    for kh2 in range(nKH2):
                        nc.tensor.matmul(
                            ps[:],
                            ht[ei][:, kh2, :, cc * P:(cc + 1) * P],
                            w2_sb[:, ei, kh2, :, :],
                            start=(kh2 == 0), stop=(kh2 == nKH2 - 1),
                            perf_mode=DR)
                    nc.vector.tensor_copy(y_sb[:, cc // 2, cc % 2, :], ps[:])
                y_tiles[ei] = y_sb

            # Stage order: every PSUM-copy / activation dependency gets a
            # full matmul stage of slack before its consumer (A0's copies
            # hide under A1, B0's activations under C0, etc).
            stage_A(0)
            stage_A(1)
            stage_B(0)
            stage_C(0)
            stage_B(1)
            stage_C(1)

            # ---- phase D: pout[T, O] = sum_e cmT_e^T y_e (fp8 DR) ----
            for mt in range(nMT):
                ps = psp.tile([P, NF], F32, tag="ps")
                idx = 0
                for ei in range(2):
                    for kp in range(nCP):
                        nc.tensor.matmul(ps[:],
                                         cmt_t[(ei, kp)][:, :, mt * P:(mt + 1) * P],
                                         y_tiles[ei][:, kp, :, :],
                                         start=(idx == 0), stop=(idx == 7),
                                         perf_mode=DR)
                        idx += 1
                ot = outp.tile([P, O], BF16, tag="out")
                # alternate copy engines: vector is busy with C1's y-copies
                # when the drain starts, which otherwise delays it ~5us
                if mt % 2 == 0:
                    nc.scalar.activation(ot[:], ps[:],
                                         mybir.ActivationFunctionType.Copy)
                else:
                    nc.vector.tensor_copy(ot[:], ps[:])
                if mt < nMT - 4:
                    nc.sync.dma_start(pout[mt * P:(mt + 1) * P, :], ot[:])
                else:
                    # last chunks: split across both queues to shorten the
                    # end-of-kernel drain chain
                    h = P // 2
                    nc.sync.dma_start(pout[mt * P:mt * P + h, :], ot[0:h, :])
                    nc.scalar.dma_start(pout[mt * P + h:(mt + 1) * P, :],
                                        ot[h:P, :])

    nc.compile()
    return nc


def get_nc():
    global _NC
    if _NC is None:
        _NC = _build()
    return _NC


def make_in_maps(x, dispatch_mask, combine_array, w1, b1, w2):
    f8 = ml_dtypes.float8_e4m3
    in_maps = []
    # x in DR plane layout [nKP, P, 2, D], t = kp*256 + i*128 + p
    x8_by_b = [
        np.ascontiguousarray(
            x[b].reshape(nKP, 2, P, D).transpose(0, 2, 1, 3)).astype(f8)
        for b in range(B)]
    w18 = w1.astype(f8)
    w28 = w2.astype(f8)
    for m in range(8):
        b, g = m // 2, m % 2
        es = slice(2 * g, 2 * g + 2)
        # dm -> [2, nKP, P, 2, C] (shifted), t = kp*256 + i*128 + p
        dm_s = np.ascontiguousarray(
            (np.transpose(dispatch_mask[b, :, es, :], (1, 0, 2)) - DM_SHIFT)
            .reshape(2, nKP, 2, P, C).transpose(0, 1, 3, 2, 4)).astype(f8)
        # cmT -> [2, nCP, P, 2, T], c = cp*256 + i*128 + p
        cmt_s = np.ascontiguousarray(
            np.transpose(combine_array[b, :, es, :], (1, 2, 0))
            .reshape(2, nCP, 2, P, T).transpose(0, 1, 3, 2, 4)).astype(f8)
        # w1 -> [P, 2, nKD2, 2, HE], d = kd2*256 + i*128 + p
        w1_s = np.ascontiguousarray(
            w18[es].reshape(2, nKD2, 2, P, HE).transpose(3, 0, 1, 2, 4))
        # w2 -> [P, 2, nKH2, 2, O], h' = kh2*256 + i*128 + p
        w2_s = np.ascontiguousarray(
            w28[es].reshape(2, nKH2, 2, P, O).transpose(3, 0, 1, 2, 4))
        # beta = w1^T v + b1 in fp32 with EXACT x and w1 (kills the rank-1
        # component of the x / w1 fp8 quantization errors)
        v = DM_SHIFT * x[b].sum(axis=0)                      # [D]
        beta = np.einsum("edh,d->eh", w1[es], v) + b1[es]    # [2, HE]
        beta_s = np.ascontiguousarray(
            beta.reshape(2, nMH, P).transpose(2, 0, 1).reshape(P, 2 * nMH))
        in_maps.append({
            "xb": x8_by_b[b],
            "dm": dm_s,
            "cmt": cmt_s,
            "w1": w1_s,
            "w2": w2_s,
            "beta": beta_s.astype(np.float32),
        })
    return in_maps


def _norm_cdf(z):
    from math import erf
    return 0.5 * (1.0 + np.array([erf(v / np.sqrt(2.0)) for v in z],
                                 dtype=np.float64))


def kernel(x, dispatch_mask, combine_array, w1, b1, w2, b2):
    nc = get_nc()
    x, dispatch_mask, combine_array, w1, b1, w2 = (
        np.asarray(a, dtype=np.float32)
        for a in (x, dispatch_mask, combine_array, w1, b1, w2))
    in_maps = make_in_maps(x, dispatch_mask, combine_array, w1, b1, w2)
    res = bass_utils.run_bass_kernel_spmd(nc, in_maps, core_ids=list(range(8)))
    b2f = np.asarray(b2, dtype=np.float32)
    f8 = ml_dtypes.float8_e4m3
    w1q = w1.astype(f8).astype(np.float32)
    w2q = w2.astype(f8).astype(np.float32)
    ew2 = w2 - w2q                                           # [E, HE, O]
    xq = x.astype(f8).astype(np.float32)
    uq = (dispatch_mask - DM_SHIFT).astype(f8).astype(np.float32)
    out = np.empty((B, T, O), dtype=np.float32)
    for b in range(B):
        p0 = np.asarray(res.results[2 * b]["pout"], dtype=np.float32)
        p1 = np.asarray(res.results[2 * b + 1]["pout"], dtype=np.float32)
        acc = p0 + p1 + b2f
        # w2-quantization rank-1 correction per expert:
        #   out += outer(rowsum_cm, mu_h @ ew2)  with mu_h = E_c[h] estimated
        # analytically: pre-GELU activations are ~N(beta, s2) across c, so
        # mu_h = E[gelu(N(beta, s2))] in closed form (Gaussian integral).
        rs_cm = combine_array[b].sum(axis=2)                 # [T, E]
        v = DM_SHIFT * x[b].sum(axis=0)
        vu = uq[b].var(axis=2)                               # [T, E]
        for e in range(E):
            beta = (w1[e].T @ v + b1[e]).astype(np.float64)  # [HE]
            s2 = (w1q[e] ** 2).T @ ((xq[b] ** 2).T @ vu[:, e])
            s2 = s2.astype(np.float64)
            zr = beta / np.sqrt(1.0 + s2)
            phi = np.exp(-0.5 * zr * zr) / np.sqrt(2.0 * np.pi)
            mu = beta * _norm_cdf(zr) + s2 / np.sqrt(1.0 + s2) * phi
            acc += np.outer(rs_cm[:, e],
                            mu.astype(np.float32) @ ew2[e])
        out[b] = acc
    return out
